# revision 1
# baseline (speedup 1.0000x reference)
"""Trainium2 Bass kernel for nn_CumulantSOAP_CV: per-column cumulants of
X (100000, 1024) up to order 5, then (X_cum - mu) @ W -> (1, 8).

Strategy (8 NeuronCores, SPMD):
  - Host pads X to 100352 rows (zeros don't perturb power sums) and shards
    rows across 8 cores: 12544 rows = 98 tiles of (128, 1024) each.
  - Per core, one pass over X: DMA-cast f32->bf16 (SWDGE), then per tile
    compute x^2 (ScalarE Square), x^3 = x*x^2 and x^5 = x^2*x^3 (VectorE).
    Column sums of x, x^2, x^3, x^5 via ones-vector matmuls accumulated in
    PSUM, issued at 4 distinct 32-col PE strips so they run concurrently.
    S4 = sum((x^2)^2) via PE "diagonal" matmuls x2_chunk^T @ x2_chunk
    (128-col chunks) accumulated in PSUM; diagonal extracted at the end.
  - 5x1024 raw power sums are AllReduduced across the 8 cores, cumulants
    combined from raw moments, and the (1,5120)@(5120,8) projection done
    replicated on every core. Output taken from core 0.
"""

import numpy as np

import concourse.bass as bass
import concourse.mybir as mybir
import concourse.tile as tile
from concourse import bacc
from concourse.bass_utils import run_bass_kernel_spmd
from concourse.masks import make_identity

N_CORES = 8
N_TRUE = 100000
P = 1024
ROWS_PER_CORE = 12544      # 98 tiles of 128
NT = ROWS_PER_CORE // 128  # 98
K_OUT = 8
SCALE = 1.0 / float(N_TRUE)

F32 = mybir.dt.float32
BF16 = mybir.dt.bfloat16
Alu = mybir.AluOpType
Act = mybir.ActivationFunctionType


import os

USE_TTR = os.environ.get("K_TTR", "1") == "1"
USE_BIG_WARM = os.environ.get("K_BIGWARM", "1") == "1"


def _build(rows_per_core=ROWS_PER_CORE, stage=5):
    nt_total = rows_per_core // 128
    nc = bacc.Bacc("TRN2", target_bir_lowering=False, debug=False,
                   num_devices=N_CORES)
    X = nc.dram_tensor("X", [rows_per_core, P], F32, kind="ExternalInput")
    MU = nc.dram_tensor("mu", [1, 5 * P], F32, kind="ExternalInput")
    W = nc.dram_tensor("W", [5 * P, K_OUT], F32, kind="ExternalInput")
    OUT = nc.dram_tensor("out", [1, K_OUT], F32, kind="ExternalOutput")

    cc_in = nc.dram_tensor("cc_in", [5 * P], F32)
    cc_out = nc.dram_tensor("cc_out", [N_CORES * 5 * P], F32,
                            addr_space="Shared")
    warm_n = 5 * P if USE_BIG_WARM else 8
    warm_in = nc.dram_tensor("warm_in", [warm_n], F32)
    warm_out = nc.dram_tensor("warm_out", [N_CORES * warm_n], F32,
                              addr_space="Shared")
    warm_ar_out = nc.dram_tensor("warm_ar_out", [warm_n], F32,
                                 addr_space="Shared")
    wk_ins = [nc.dram_tensor(f"wk_in{i}", [8], BF16)
              for i in range(4)]
    wk_outs = [nc.dram_tensor(f"wk_out{i}", [N_CORES * 8], BF16,
                              addr_space="Shared")
               for i in range(4)]

    # DMA blocks of 4 tiles; compute sub-blocks of 4 tiles
    dbs = [(i, min(4, nt_total - i)) for i in range(0, nt_total, 4)]
    NDIAG = 6                 # S4 via PE diag for cols < NDIAG*128
    X4LO = NDIAG * 128        # S4 via ACT x^4 + plain reduce for cols >= 768
    X4W = P - X4LO            # 256
    # sub-block whose power tile gates a cc keep-warm op (the cc path
    # re-cools during long idle gaps). ONE gate only: each warm op runs
    # 22-30us, so a chain of them queues ahead of the real AllGather and
    # eats the saving. A single gate ending just before the real trigger
    # (~194us) keeps the path warm with no queue.
    WARM_GATE_SUBBLOCKS = (38,)

    with tile.TileContext(nc) as tc:
        with (
            tc.tile_pool(name="xin", bufs=6) as xin,
            tc.tile_pool(name="pows", bufs=4) as pows,
            tc.tile_pool(name="const", bufs=1) as const,
            tc.tile_pool(name="acc", bufs=1, space="PSUM") as accp,
            tc.tile_pool(name="tailps", bufs=1, space="PSUM") as tailps,
            tc.tile_pool(name="tail", bufs=1) as tailp,
        ):
            Xv = X.ap()

            # issue the first X block's DMA before any constant setup so
            # the HBM stream starts immediately
            t0_0, ndt_0 = dbs[0]
            x_first = xin.tile([128, ndt_0, P], BF16, tag="x")
            nh0 = max(1, ndt_0 // 2)
            for lo in range(0, ndt_0, nh0):
                w = min(nh0, ndt_0 - lo)
                nc.gpsimd.dma_start(
                    out=x_first[:, lo:lo + w, :],
                    in_=Xv[(t0_0 + lo) * 128:(t0_0 + lo + w) * 128, :].rearrange(
                        "(p s) c -> p s c", s=w),
                )

            ones = const.tile([128, 1], BF16)
            nc.vector.memset(ones, 1.0)
            ident = const.tile([128, 128], F32)
            make_identity(nc, ident)
            # identity replicated along free axis for one-shot S4 extract
            ident_rep = const.tile([128, NDIAG, 128], F32)
            for c in range(NDIAG):
                nc.vector.tensor_copy(ident_rep[:, c, :], ident)

            # warm-up collectives at the REAL payload size, launched
            # immediately: they absorb the ncfw/TOPSP cold-start barrier
            # and pre-establish the 20KB slicing under the main loop so
            # the real AllGather at the tail runs as warm as possible.
            wtile = const.tile([1, 8], F32)
            nc.vector.memset(wtile, 0.0)
            nc.sync.dma_start(out=warm_in.ap()[0:8], in_=wtile[0:1, :])
            nc.gpsimd.collective_compute(
                "AllReduce", Alu.add,
                replica_groups=[list(range(N_CORES))],
                ins=[warm_in.ap()],
                outs=[warm_ar_out.ap()],
            )
            nc.gpsimd.collective_compute(
                "AllGather", Alu.bypass,
                replica_groups=[list(range(N_CORES))],
                ins=[warm_in.ap()],
                outs=[warm_out.ap()],
            )

            # weights / mu for the tail (contraction row j5 = 40*p + q)
            w_s = const.tile([128, 40, K_OUT], F32)
            nc.sync.dma_start(out=w_s, in_=W.ap().rearrange(
                "(p q) k -> p q k", p=128))
            mu_s = const.tile([128, 40], F32)
            nc.sync.dma_start(out=mu_s, in_=MU.ap()[0, :].rearrange(
                "(p q) -> p q", p=128))

            # PSUM accumulators, alive across the whole main loop
            ps_plain = accp.tile([128, P], F32)   # S1@p0 S2@p32 S3@p64 S5@p96
            ps_diag = accp.tile([128, X4LO], F32)  # S4 diag blocks, cols<640
            if X4W:
                ps_x4 = accp.tile([1, X4W], F32, name="ps_x4")
            else:
                ps_x4 = None

            for bi, (t0, ndt) in enumerate(dbs):
                if bi == 0:
                    x = x_first
                else:
                    x = xin.tile([128, ndt, P], BF16, tag="x")
                    # partition p holds CONSECUTIVE rows -> contiguous
                    # multi-KB DMA runs per partition (row->partition
                    # placement is free for column sums). Two DMAs per
                    # block for finer arrival.
                    nh = max(1, ndt // 2)
                    for lo in range(0, ndt, nh):
                        w = min(nh, ndt - lo)
                        nc.gpsimd.dma_start(
                            out=x[:, lo:lo + w, :],
                            in_=Xv[(t0 + lo) * 128:(t0 + lo + w) * 128, :].rearrange(
                                "(p s) c -> p s c", s=w),
                        )
                for half in range(0, ndt, 2):
                    nt = min(2, ndt - half)
                    xh = x[:, half:half + nt, :]
                    sq = pows.tile([128, nt, P], BF16, tag="sq")
                    nc.scalar.activation(sq, xh, Act.Square)
                    cu = pows.tile([128, nt, P], BF16, tag="cu")
                    nc.vector.tensor_mul(cu, xh, sq)
                    x5 = pows.tile([128, nt, P], BF16, tag="x5")
                    nc.vector.tensor_mul(x5, sq, cu)
                    if X4W:
                        x4 = pows.tile([128, nt, X4W], BF16, tag="x4")
                        nc.scalar.activation(x4, sq[:, :, X4LO:], Act.Square)

                    sb = (t0 + half) // 2
                    if sb in WARM_GATE_SUBBLOCKS:
                        # cc keep-warm: a tiny AllGather gated on this
                        # sub-block's x5 tile, so the cc path stays hot
                        # until the real AllGather fires. Dedicated
                        # buffers per gate; pows bufs recycle fast so the
                        # extra reader never stalls the pipeline.
                        gi = 1 + WARM_GATE_SUBBLOCKS.index(sb)
                        nc.sync.dma_start(out=wk_ins[gi].ap(),
                                          in_=x5[0:1, 0, 0:8])
                        nc.gpsimd.collective_compute(
                            "AllGather", Alu.bypass,
                            replica_groups=[list(range(N_CORES))],
                            ins=[wk_ins[gi].ap()],
                            outs=[wk_outs[gi].ap()],
                        )

                    gt0 = t0 + half
                    start = gt0 == 0
                    stop = gt0 + nt == nt_total
                    # S4 head columns first: diag blocks x2_chunk^T@x2_chunk
                    # depend only on sq, giving PE work while the DVE
                    # cu->x5 chain for this sub-block is still running.
                    # start/stop only on first/last matmul per PSUM bank
                    # (chunks 0-3 -> bank 0, chunk 4 -> bank 1)
                    for t in range(nt):
                        gt = t0 + half + t
                        st = gt == 0
                        sp = gt == nt_total - 1
                        for c in range(NDIAG):
                            cs = slice(c * 128, (c + 1) * 128)
                            nc.tensor.matmul(
                                ps_diag[:, cs], sq[:, t, cs], sq[:, t, cs],
                                start=st and c % 4 == 0,
                                stop=sp and (c == 3 or c == NDIAG - 1),
                                tile_position=(0, 0),
                            )
                    for t in range(nt):
                        gt = t0 + half + t
                        st = gt == 0
                        sp = gt == nt_total - 1
                        # plain col-sums: 4 powers on 4 concurrent col-strips
                        for h in range(2):
                            sl = slice(h * 512, (h + 1) * 512)
                            for j, pw in enumerate((xh, sq, cu, x5)):
                                bp = 32 * j
                                nc.tensor.matmul(
                                    ps_plain[bp:bp + 1, sl], ones[:, 0:1],
                                    pw[:, t, sl],
                                    start=st, stop=sp,
                                    tile_position=(0, bp),
                                )
                        if X4W:
                            # S4 tail columns: plain reduce of x^4
                            nc.tensor.matmul(
                                ps_x4[0:1, :], ones[:, 0:1], x4[:, t, :],
                                start=st, stop=sp, tile_position=(0, 0),
                            )

            # ---- tail ----
            # PSUM->SBUF scaled row copies, split across DVE and ACT so they
            # run in parallel (ACT is otherwise idle in the tail)
            srows = tailp.tile([128, P], F32)
            for jj in (0, 1):
                r = slice(32 * jj, 32 * jj + 1)
                nc.vector.tensor_scalar_mul(srows[r, :], ps_plain[r, :], SCALE)
            for jj in (2, 3):
                r = slice(32 * jj, 32 * jj + 1)
                nc.scalar.activation(srows[r, :], ps_plain[r, :], Act.Copy,
                                     scale=SCALE)

            if stage >= 2:
                s4_s = tailp.tile([128, NDIAG], F32)
                dummy = tailp.tile([128, NDIAG, 128], F32)
                nc.vector.scalar_tensor_tensor(
                    dummy, ps_diag[:].rearrange("p (c i) -> p c i", i=128),
                    SCALE, ident_rep, Alu.mult, Alu.mult)
                nc.vector.tensor_reduce(
                    s4_s, dummy, axis=mybir.AxisListType.X, op=Alu.add)
                if X4W:
                    s4row = tailp.tile([1, X4W], F32)
                    nc.scalar.activation(s4row, ps_x4, Act.Copy, scale=SCALE)

            if stage >= 3:
                # stage scaled raw moments to DRAM: [M1|M2|M3|M4|M5] by
                # column. HWDGE (sync/scalar) for low fixed latency.
                for jj, k in ((0, 0), (1, 1), (2, 2), (3, 4)):
                    eng = nc.sync if jj % 2 == 0 else nc.scalar
                    eng.dma_start(
                        out=cc_in.ap()[k * P:(k + 1) * P],
                        in_=srows[32 * jj:32 * jj + 1, :],
                    )
                nc.sync.dma_start(
                    out=cc_in.ap()[3 * P:3 * P + X4LO].rearrange(
                        "(c i) -> i c", i=128),
                    in_=s4_s,
                )
                if X4W:
                    nc.scalar.dma_start(
                        out=cc_in.ap()[3 * P + X4LO:4 * P],
                        in_=s4row[0:1, :],
                    )

                nc.gpsimd.collective_compute(
                    "AllGather", Alu.bypass,
                    replica_groups=[list(range(N_CORES))],
                    ins=[cc_in.ap()],
                    outs=[cc_out.ap()],
                )

                # gathered per-core moments -> sum over cores on DVE
                # momg[p, k, core, cc] = cc_out[core*5120 + k*1024 + 8p + cc]
                momg = tailp.tile([128, 5, N_CORES, K_OUT], F32)
                ccv = cc_out.ap().rearrange(
                    "(r k p c) -> p k r c", r=N_CORES, k=5, p=128)
                for k in range(5):
                    eng = nc.sync if k % 2 == 0 else nc.scalar
                    eng.dma_start(out=momg[:, k, :, :], in_=ccv[:, k, :, :])
                nc.vector.tensor_add(momg[:, :, 0:4, :], momg[:, :, 0:4, :],
                                     momg[:, :, 4:8, :])
                nc.vector.tensor_add(momg[:, :, 0:2, :], momg[:, :, 0:2, :],
                                     momg[:, :, 2:4, :])
                nc.vector.tensor_add(momg[:, :, 0:1, :], momg[:, :, 0:1, :],
                                     momg[:, :, 1:2, :])
                # global moments view, (128, 5, 8): [p, k, cc]
                mom = momg[:, :, 0, :]

            if stage >= 4:
                m = mom[:, 0, :]
                M2 = mom[:, 1, :]
                M3 = mom[:, 2, :]
                M4 = mom[:, 3, :]
                M5 = mom[:, 4, :]

                stt = nc.vector.scalar_tensor_tensor
                scr = tailp.tile([128, 12, 8], F32)  # scratch (128,8) slots
                m2, m3, m5, a2, a3, a4, mu2, mu3, b1, c3, c4, t1 = (
                    scr[:, i, :] for i in range(12))

                # cumulants written straight into interleaved v slices:
                # v[p, 5*cc + k] = c_k(col 8p+cc)
                v = tailp.tile([128, 40], F32)
                vv = v[:].rearrange("p (c k) -> p c k", k=5)

                nc.vector.tensor_mul(m2, m, m)                   # m^2
                nc.vector.tensor_mul(m3, m2, m)                  # m^3
                nc.vector.tensor_mul(m5, m2, m3)                 # m^5
                nc.vector.tensor_sub(mu2, M2, m2)                # mu2 = M2-m^2
                nc.vector.tensor_copy(vv[:, :, 0], m)
                nc.vector.memset(vv[:, :, 1], 0.0)
                nc.vector.tensor_copy(vv[:, :, 2], mu2)
                # mu3 = M3 + (-3 M2)*m + 2 m^3
                stt(b1, M2, -3.0, m, Alu.mult, Alu.mult)         # -3 m M2
                nc.vector.tensor_add(b1, b1, M3)
                stt(mu3, m3, 2.0, b1, Alu.mult, Alu.add)         # +2m^3
                # c3 = mu3 - 3 mu2^2
                stt(c3, mu2, -3.0, mu2, Alu.mult, Alu.mult)
                nc.vector.tensor_add(vv[:, :, 3], c3, mu3)
                # mu5 = M5 - 5 m M4 + 10 m^2 M3 - 10 m^3 M2 + 4 m^5
                stt(a4, M4, -5.0, m, Alu.mult, Alu.mult)
                stt(a3, M3, 10.0, m2, Alu.mult, Alu.mult)
                stt(a2, M2, -10.0, m3, Alu.mult, Alu.mult)
                nc.vector.tensor_add(a4, a4, M5)
                stt(a3, m5, 4.0, a3, Alu.mult, Alu.add)
                nc.vector.tensor_add(a4, a4, a3)
                nc.vector.tensor_add(a4, a4, a2)                 # mu5
                # c4 = mu5 - 10 mu2 mu3
                stt(t1, mu2, -10.0, mu3, Alu.mult, Alu.mult)
                nc.vector.tensor_add(vv[:, :, 4], a4, t1)

                nc.vector.tensor_sub(v, v, mu_s)

            if stage >= 5 and USE_TTR:
                # projection: collapse q on DVE (k-major scratch so one
                # X-axis reduce yields (128, K)), then a single matmul
                # collapses the partition axis.
                wv = w_s[:].rearrange("p q k -> p k q")
                prod = tailp.tile([128, K_OUT, 40], F32)
                for k in range(K_OUT):
                    nc.vector.tensor_mul(prod[:, k, :], v, wv[:, k, :])
                colk = tailp.tile([128, K_OUT], F32)
                nc.vector.tensor_reduce(colk, prod,
                                        axis=mybir.AxisListType.X, op=Alu.add)
                ps_out = tailps.tile([1, K_OUT], F32)
                ones_f = tailp.tile([128, 1], F32)
                nc.vector.memset(ones_f, 1.0)
                nc.tensor.matmul(ps_out[0:1, :], ones_f[:, 0:1], colk,
                                 start=True, stop=True)
                o_s = tailp.tile([1, K_OUT], F32)
                nc.vector.tensor_copy(o_s, ps_out)
                nc.sync.dma_start(out=OUT.ap(), in_=o_s)
            elif stage >= 5:
                ps_out = tailps.tile([1, K_OUT], F32)
                for q in range(40):
                    nc.tensor.matmul(
                        ps_out[0:1, :], v[:, q:q + 1], w_s[:, q, :],
                        start=(q == 0), stop=(q == 39),
                    )
                o_s = tailp.tile([1, K_OUT], F32)
                nc.vector.tensor_copy(o_s, ps_out)
                nc.sync.dma_start(out=OUT.ap(), in_=o_s)
            else:
                nc.sync.dma_start(out=OUT.ap(), in_=srows[0:1, 0:K_OUT])

    nc.compile()
    return nc


_NC = None


def _get_nc():
    global _NC
    if _NC is None:
        _NC = _build()
    return _NC


def _shard(X, mu, W):
    Xp = np.zeros((N_CORES * ROWS_PER_CORE, P), dtype=np.float32)
    Xp[:X.shape[0]] = X
    return [
        {
            "X": np.ascontiguousarray(Xp[i * ROWS_PER_CORE:(i + 1) * ROWS_PER_CORE]),
            "mu": np.ascontiguousarray(mu.astype(np.float32)),
            "W": np.ascontiguousarray(W.astype(np.float32)),
        }
        for i in range(N_CORES)
    ]


def run(X, mu, W, trace=False, **trace_kwargs):
    nc = _get_nc()
    in_maps = _shard(np.asarray(X, dtype=np.float32), np.asarray(mu),
                     np.asarray(W))
    res = run_bass_kernel_spmd(nc, in_maps, core_ids=list(range(N_CORES)),
                               trace=trace, **trace_kwargs)
    return res


def kernel(X, mu, W):
    res = run(X, mu, W, trace=False)
    return np.asarray(res.results[0]["out"], dtype=np.float32)



# revision 7
# speedup vs baseline: 1.0448x; 1.0448x over previous
"""Trainium2 Bass kernel for nn_CumulantSOAP_CV: per-column cumulants of
X (100000, 1024) up to order 5, then (X_cum - mu) @ W -> (1, 8).

Strategy (8 NeuronCores, SPMD):
  - Host casts X to bf16 (same rounding the old device-side DMA cast did)
    and pads to 100352 rows; shards rows across 8 cores: 12544 rows =
    98 tiles of (128, 1024) each. bf16 halves the HBM stream (the f32
    version was DMA-bound at ~2.9us/sub-block).
  - Per core, one pass over X: per 2-tile sub-block compute x^2 (ScalarE
    Square), x^3 = x*x^2 (DVE), x^5 = x^2*x^3 (DVE, with an optional
    column-slice offloaded to GpSimd). Column sums of x, x^2, x^3, x^5
    via ones-vector matmuls accumulated in PSUM at 4 distinct 32-col PE
    positions (concurrent strips). S4 = sum((x^2)^2) via PE "diagonal"
    matmuls x2_chunk^T @ x2_chunk for all 8 chunks, accumulated in PSUM.
  - The cross-core reduction is split in two AllGathers so the expensive
    collective latency overlaps the main loop: plain sums over tiles
    [0, PHASE_A_TILES) are DMA'd straight from PSUM to DRAM mid-loop and
    AllGather'd while the loop continues; the remainder (plus the S4
    diagonal, extracted once at the end) rides a second small AllGather
    at loop end. Each core sums the 16 partial vectors, forms cumulants,
    and does the (1,5120)@(5120,8) projection replicated. Output from
    core 0.
"""

import os

import numpy as np
import ml_dtypes

import concourse.bass as bass
import concourse.mybir as mybir
import concourse.tile as tile
from concourse import bacc
from concourse.bass_utils import run_bass_kernel_spmd
from concourse.masks import make_identity

N_CORES = 8
N_TRUE = 100000
P = 1024
ROWS_PER_CORE = 12544      # 98 tiles of 128
NT = ROWS_PER_CORE // 128  # 98
K_OUT = 8
SCALE = 1.0 / float(N_TRUE)

F32 = mybir.dt.float32
BF16 = mybir.dt.bfloat16
Alu = mybir.AluOpType
Act = mybir.ActivationFunctionType

# columns of x^5 computed on GpSimd instead of DVE (0 disables)
GP_COLS = int(os.environ.get("K_GPCOLS", "256"))
# tiles contributing to the early (overlapped) AllGather
PHASE_A_TILES = int(os.environ.get("K_PHASEA", "40"))
NDIAG = 8


def _build(rows_per_core=ROWS_PER_CORE):
    nt_total = rows_per_core // 128
    pa = PHASE_A_TILES
    nc = bacc.Bacc("TRN2", target_bir_lowering=False, debug=False,
                   num_devices=N_CORES)
    X = nc.dram_tensor("X", [rows_per_core, P], BF16, kind="ExternalInput")
    MU = nc.dram_tensor("mu", [1, 5 * P], F32, kind="ExternalInput")
    W = nc.dram_tensor("W", [5 * P, K_OUT], F32, kind="ExternalInput")
    OUT = nc.dram_tensor("out", [1, K_OUT], F32, kind="ExternalOutput")

    # phase A payload: [S1|S2|S3|S5] raw sums over tiles [0, pa)
    ccA_in = nc.dram_tensor("ccA_in", [4 * P], F32)
    ccA_out = nc.dram_tensor("ccA_out", [N_CORES * 4 * P], F32,
                             addr_space="Shared")
    # phase B payload: [S1|S2|S3|S5|S4] (S4 over ALL tiles, rest over
    # tiles [pa, nt_total))
    ccB_in = nc.dram_tensor("ccB_in", [5 * P], F32)
    ccB_out = nc.dram_tensor("ccB_out", [N_CORES * 5 * P], F32,
                             addr_space="Shared")
    warm_in = nc.dram_tensor("warm_in", [5 * P], F32)
    warm_out = nc.dram_tensor("warm_out", [N_CORES * 5 * P], F32,
                              addr_space="Shared")

    # DMA blocks of 4 tiles; compute sub-blocks of 2 tiles
    dbs = [(i, min(4, nt_total - i)) for i in range(0, nt_total, 4)]

    with tile.TileContext(nc) as tc:
        with (
            tc.tile_pool(name="xin", bufs=6) as xin,
            tc.tile_pool(name="pows", bufs=4) as pows,
            tc.tile_pool(name="const", bufs=1) as const,
            tc.tile_pool(name="acc", bufs=1, space="PSUM") as accp,
            tc.tile_pool(name="tailps", bufs=1, space="PSUM") as tailps,
            tc.tile_pool(name="tail", bufs=1) as tailp,
        ):
            Xv = X.ap()

            # issue the first X block's DMA before any constant setup so
            # the HBM stream starts immediately
            t0_0, ndt_0 = dbs[0]
            x_first = xin.tile([128, ndt_0, P], BF16, tag="x")
            nh0 = max(1, ndt_0 // 2)
            for lo in range(0, ndt_0, nh0):
                w = min(nh0, ndt_0 - lo)
                nc.sync.dma_start(
                    out=x_first[:, lo:lo + w, :],
                    in_=Xv[(t0_0 + lo) * 128:(t0_0 + lo + w) * 128, :].rearrange(
                        "(p s) c -> p s c", s=w),
                )

            ones = const.tile([128, 1], BF16)
            nc.vector.memset(ones, 1.0)
            ident = const.tile([128, 128], F32)
            make_identity(nc, ident)
            # identity replicated along free axis for one-shot S4 extract
            ident_rep = const.tile([128, NDIAG, 128], F32)
            for c in range(NDIAG):
                nc.vector.tensor_copy(ident_rep[:, c, :], ident)

            # warm-up collective at the real payload size, launched
            # immediately: absorbs the ncfw/TOPSP cold-start barrier.
            wtile = const.tile([1, 8], F32)
            nc.vector.memset(wtile, 0.0)
            nc.scalar.dma_start(out=warm_in.ap()[0:8], in_=wtile[0:1, :])
            nc.gpsimd.collective_compute(
                "AllGather", Alu.bypass,
                replica_groups=[list(range(N_CORES))],
                ins=[warm_in.ap()],
                outs=[warm_out.ap()],
            )

            # weights / mu for the tail (contraction row j5 = 40*p + q)
            w_s = const.tile([128, 40, K_OUT], F32)
            nc.scalar.dma_start(out=w_s, in_=W.ap().rearrange(
                "(p q) k -> p q k", p=128))
            mu_s = const.tile([128, 40], F32)
            nc.scalar.dma_start(out=mu_s, in_=MU.ap()[0, :].rearrange(
                "(p q) -> p q", p=128))

            # PSUM accumulators, alive across the whole main loop.
            # ps_plain: S1@p0 S2@p32 S3@p64 S5@p96, reused by both phases
            # (phase A is DMA'd out mid-loop, then phase B restarts with
            # start=True). ps_diag accumulates S4 over ALL tiles.
            ps_plain = accp.tile([128, P], F32)
            ps_diag = accp.tile([128, NDIAG * 128], F32)

            for bi, (t0, ndt) in enumerate(dbs):
                if bi == 0:
                    x = x_first
                else:
                    x = xin.tile([128, ndt, P], BF16, tag="x")
                    # partition p holds CONSECUTIVE rows -> contiguous
                    # multi-KB DMA runs per partition (row->partition
                    # placement is free for column sums). Two DMAs per
                    # block for finer arrival.
                    nh = max(1, ndt // 2)
                    for lo in range(0, ndt, nh):
                        w = min(nh, ndt - lo)
                        nc.sync.dma_start(
                            out=x[:, lo:lo + w, :],
                            in_=Xv[(t0 + lo) * 128:(t0 + lo + w) * 128, :].rearrange(
                                "(p s) c -> p s c", s=w),
                        )
                for half in range(0, ndt, 2):
                    nt = min(2, ndt - half)
                    xh = x[:, half:half + nt, :]
                    sq = pows.tile([128, nt, P], BF16, tag="sq")
                    nc.scalar.activation(sq, xh, Act.Square)
                    cu = pows.tile([128, nt, P], BF16, tag="cu")
                    nc.vector.tensor_mul(cu, xh, sq)
                    x5 = pows.tile([128, nt, P], BF16, tag="x5")
                    if GP_COLS:
                        dv = P - GP_COLS
                        nc.vector.tensor_mul(x5[:, :, :dv], sq[:, :, :dv],
                                             cu[:, :, :dv])
                        nc.gpsimd.tensor_mul(x5[:, :, dv:], sq[:, :, dv:],
                                             cu[:, :, dv:])
                    else:
                        nc.vector.tensor_mul(x5, sq, cu)

                    for t in range(nt):
                        gt = t0 + half + t
                        st = gt == 0
                        sp = gt == nt_total - 1
                        # S4 diag blocks x2_chunk^T @ x2_chunk: depend
                        # only on sq; single phase across the whole loop.
                        # start/stop per PSUM bank (chunks 0-3 -> bank 0,
                        # 4-7 -> bank 1).
                        for c in range(NDIAG):
                            cs = slice(c * 128, (c + 1) * 128)
                            nc.tensor.matmul(
                                ps_diag[:, cs], sq[:, t, cs], sq[:, t, cs],
                                start=st and c % 4 == 0,
                                stop=sp and (c == 3 or c == NDIAG - 1),
                                tile_position=(0, 0),
                            )
                        # plain col-sums: 4 powers on 4 concurrent
                        # col-strips; two accumulation phases.
                        stp = gt == 0 or gt == pa
                        spp = gt == pa - 1 or gt == nt_total - 1
                        for h in range(2):
                            sl = slice(h * 512, (h + 1) * 512)
                            for j, pw in enumerate((xh, sq, cu, x5)):
                                bp = 32 * j
                                nc.tensor.matmul(
                                    ps_plain[bp:bp + 1, sl], ones[:, 0:1],
                                    pw[:, t, sl],
                                    start=stp, stop=spp,
                                    tile_position=(0, bp),
                                )

                    # phase A readout + early AllGather: one strided
                    # PSUM -> SBUF copy (rows 0/32/64/96 -> 4 partitions)
                    # + one DMA, then the collective overlaps the rest
                    # of the main loop.
                    if t0 + half + nt == pa:
                        rowsA = tailp.tile([128, P], F32, name="rowsA")
                        nc.scalar.activation(rowsA, ps_plain, Act.Copy)
                        for jj in range(4):
                            nc.scalar.dma_start(
                                out=ccA_in.ap()[jj * P:(jj + 1) * P],
                                in_=rowsA[32 * jj:32 * jj + 1, :],
                            )
                        nc.gpsimd.collective_compute(
                            "AllGather", Alu.bypass,
                            replica_groups=[list(range(N_CORES))],
                            ins=[ccA_in.ap()],
                            outs=[ccA_out.ap()],
                        )

            # ---- tail ----
            # S4 diagonal extract: (ps_diag * ident_rep) then reduce the
            # 128-wide X axis -> (128, NDIAG) raw sums.
            s4_s = tailp.tile([128, NDIAG], F32)
            dummy = tailp.tile([128, NDIAG, 128], F32)
            nc.vector.scalar_tensor_tensor(
                dummy, ps_diag[:].rearrange("p (c i) -> p c i", i=128),
                1.0, ident_rep, Alu.mult, Alu.mult)
            nc.vector.tensor_reduce(
                s4_s, dummy, axis=mybir.AxisListType.X, op=Alu.add)

            # phase B payload: plain rows via one full-tile copy, S4 from SBUF
            rowsB = tailp.tile([128, P], F32, name="rowsB")
            nc.scalar.activation(rowsB, ps_plain, Act.Copy)
            for jj in range(4):
                eng = nc.scalar if jj % 2 == 0 else nc.sync
                eng.dma_start(
                    out=ccB_in.ap()[jj * P:(jj + 1) * P],
                    in_=rowsB[32 * jj:32 * jj + 1, :],
                )
            nc.sync.dma_start(
                out=ccB_in.ap()[4 * P:5 * P].rearrange("(c i) -> i c", i=128),
                in_=s4_s,
            )
            nc.gpsimd.collective_compute(
                "AllGather", Alu.bypass,
                replica_groups=[list(range(N_CORES))],
                ins=[ccB_in.ap()],
                outs=[ccB_out.ap()],
            )

            # gathered per-core partials -> summed raw moments.
            # momg*[p, k, core, cc]; column j of moment k is (8p + cc).
            ga = tailp.tile([128, 4, N_CORES, K_OUT], F32)
            gb = tailp.tile([128, 5, N_CORES, K_OUT], F32)
            ccAv = ccA_out.ap().rearrange(
                "(r k p c) -> p k r c", r=N_CORES, k=4, p=128)
            ccBv = ccB_out.ap().rearrange(
                "(r k p c) -> p k r c", r=N_CORES, k=5, p=128)
            for k in range(4):
                eng = nc.sync if k % 2 == 0 else nc.scalar
                eng.dma_start(out=ga[:, k, :, :], in_=ccAv[:, k, :, :])
            for k in range(5):
                eng = nc.scalar if k % 2 == 0 else nc.sync
                eng.dma_start(out=gb[:, k, :, :], in_=ccBv[:, k, :, :])
            for g, kk in ((ga, 4), (gb, 5)):
                nc.vector.tensor_add(g[:, :, 0:4, :], g[:, :, 0:4, :],
                                     g[:, :, 4:8, :])
                nc.vector.tensor_add(g[:, :, 0:2, :], g[:, :, 0:2, :],
                                     g[:, :, 2:4, :])
                nc.vector.tensor_add(g[:, :, 0:1, :], g[:, :, 0:1, :],
                                     g[:, :, 1:2, :])
            # smom[p, k, cc], k in [M1,M2,M3,M5,M4] order; scaled by 1/N
            smom = tailp.tile([128, 5, K_OUT], F32)
            nc.vector.tensor_add(smom[:, 0:4, :], ga[:, :, 0, :],
                                 gb[:, 0:4, 0, :])
            nc.vector.tensor_copy(smom[:, 4, :], gb[:, 4, 0, :])
            nc.vector.tensor_scalar_mul(smom, smom, SCALE)

            m = smom[:, 0, :]
            M2 = smom[:, 1, :]
            M3 = smom[:, 2, :]
            M5 = smom[:, 3, :]
            M4 = smom[:, 4, :]

            stt = nc.vector.scalar_tensor_tensor
            scr = tailp.tile([128, 12, 8], F32)  # scratch (128,8) slots
            m2, m3, m5, a2, a3, a4, mu2, mu3, b1, c3, c4, t1 = (
                scr[:, i, :] for i in range(12))

            # cumulants written straight into interleaved v slices:
            # v[p, 5*cc + k] = c_k(col 8p+cc)
            v = tailp.tile([128, 40], F32)
            vv = v[:].rearrange("p (c k) -> p c k", k=5)

            nc.vector.tensor_mul(m2, m, m)                   # m^2
            nc.vector.tensor_mul(m3, m2, m)                  # m^3
            nc.vector.tensor_mul(m5, m2, m3)                 # m^5
            nc.vector.tensor_sub(mu2, M2, m2)                # mu2 = M2-m^2
            nc.vector.tensor_copy(vv[:, :, 0], m)
            nc.vector.memset(vv[:, :, 1], 0.0)
            nc.vector.tensor_copy(vv[:, :, 2], mu2)
            # mu3 = M3 + (-3 M2)*m + 2 m^3
            stt(b1, M2, -3.0, m, Alu.mult, Alu.mult)         # -3 m M2
            nc.vector.tensor_add(b1, b1, M3)
            stt(mu3, m3, 2.0, b1, Alu.mult, Alu.add)         # +2m^3
            # c3 = mu3 - 3 mu2^2
            stt(c3, mu2, -3.0, mu2, Alu.mult, Alu.mult)
            nc.vector.tensor_add(vv[:, :, 3], c3, mu3)
            # mu5 = M5 - 5 m M4 + 10 m^2 M3 - 10 m^3 M2 + 4 m^5
            stt(a4, M4, -5.0, m, Alu.mult, Alu.mult)
            stt(a3, M3, 10.0, m2, Alu.mult, Alu.mult)
            stt(a2, M2, -10.0, m3, Alu.mult, Alu.mult)
            nc.vector.tensor_add(a4, a4, M5)
            stt(a3, m5, 4.0, a3, Alu.mult, Alu.add)
            nc.vector.tensor_add(a4, a4, a3)
            nc.vector.tensor_add(a4, a4, a2)                 # mu5
            # c4 = mu5 - 10 mu2 mu3
            stt(t1, mu2, -10.0, mu3, Alu.mult, Alu.mult)
            nc.vector.tensor_add(vv[:, :, 4], a4, t1)

            nc.vector.tensor_sub(v, v, mu_s)

            # projection: collapse q on DVE (k-major scratch so one
            # X-axis reduce yields (128, K)), then a single matmul
            # collapses the partition axis.
            wv = w_s[:].rearrange("p q k -> p k q")
            prod = tailp.tile([128, K_OUT, 40], F32)
            for k in range(K_OUT):
                nc.vector.tensor_mul(prod[:, k, :], v, wv[:, k, :])
            colk = tailp.tile([128, K_OUT], F32)
            nc.vector.tensor_reduce(colk, prod,
                                    axis=mybir.AxisListType.X, op=Alu.add)
            ps_out = tailps.tile([1, K_OUT], F32)
            ones_f = tailp.tile([128, 1], F32)
            nc.vector.memset(ones_f, 1.0)
            nc.tensor.matmul(ps_out[0:1, :], ones_f[:, 0:1], colk,
                             start=True, stop=True)
            o_s = tailp.tile([1, K_OUT], F32)
            nc.vector.tensor_copy(o_s, ps_out)
            nc.sync.dma_start(out=OUT.ap(), in_=o_s)

    nc.compile()
    return nc


_NC = None


def _get_nc():
    global _NC
    if _NC is None:
        _NC = _build()
    return _NC


def _shard(X, mu, W):
    Xb = np.asarray(X, dtype=np.float32).astype(ml_dtypes.bfloat16)
    Xp = np.zeros((N_CORES * ROWS_PER_CORE, P), dtype=ml_dtypes.bfloat16)
    Xp[:Xb.shape[0]] = Xb
    return [
        {
            "X": np.ascontiguousarray(Xp[i * ROWS_PER_CORE:(i + 1) * ROWS_PER_CORE]),
            "mu": np.ascontiguousarray(mu.astype(np.float32)),
            "W": np.ascontiguousarray(W.astype(np.float32)),
        }
        for i in range(N_CORES)
    ]


def run(X, mu, W, trace=False, **trace_kwargs):
    nc = _get_nc()
    in_maps = _shard(X, np.asarray(mu), np.asarray(W))
    res = run_bass_kernel_spmd(nc, in_maps, core_ids=list(range(N_CORES)),
                               trace=trace, **trace_kwargs)
    return res


def kernel(X, mu, W):
    res = run(X, mu, W, trace=False)
    return np.asarray(res.results[0]["out"], dtype=np.float32)


# revision 9
# speedup vs baseline: 1.1953x; 1.1440x over previous
"""Trainium2 Bass kernel for nn_CumulantSOAP_CV: per-column cumulants of
X (100000, 1024) up to order 5, then (X_cum - mu) @ W -> (1, 8).

Strategy (8 NeuronCores, SPMD):
  - Host casts X to bf16 (same rounding the old device-side DMA cast did)
    and pads to 100352 rows; shards rows across 8 cores: 12544 rows =
    98 tiles of (128, 1024) each. bf16 halves the HBM stream (the f32
    version was DMA-bound at ~2.9us/sub-block).
  - Per core, one pass over X: per 2-tile sub-block compute x^2 (ScalarE
    Square), x^3 = x*x^2 (DVE), x^5 = x^2*x^3 (DVE, with an optional
    column-slice offloaded to GpSimd). Column sums of x, x^2, x^3, x^5
    via ones-vector matmuls accumulated in PSUM at 4 distinct 32-col PE
    positions (concurrent strips). S4 = sum((x^2)^2) via PE "diagonal"
    matmuls x2_chunk^T @ x2_chunk for all 8 chunks, accumulated in PSUM.
  - The cross-core reduction is split in two AllGathers so the expensive
    collective latency overlaps the main loop: plain sums over tiles
    [0, PHASE_A_TILES) are DMA'd straight from PSUM to DRAM mid-loop and
    AllGather'd while the loop continues; the remainder (plus the S4
    diagonal, extracted once at the end) rides a second small AllGather
    at loop end. Each core sums the 16 partial vectors, forms cumulants,
    and does the (1,5120)@(5120,8) projection replicated. Output from
    core 0.
"""

import os

import numpy as np
import ml_dtypes

import concourse.bass as bass
import concourse.mybir as mybir
import concourse.tile as tile
from concourse import bacc
from concourse.bass_utils import run_bass_kernel_spmd
from concourse.masks import make_identity

N_CORES = 8
N_TRUE = 100000
P = 1024
ROWS_PER_CORE = 12544      # 98 tiles of 128
NT = ROWS_PER_CORE // 128  # 98
K_OUT = 8
SCALE = 1.0 / float(N_TRUE)

F32 = mybir.dt.float32
BF16 = mybir.dt.bfloat16
Alu = mybir.AluOpType
Act = mybir.ActivationFunctionType

# columns of x^5 computed on GpSimd instead of DVE (0 disables; >0 measured
# HARMFUL: GpSimd SBUF traffic knocks the DVE cu op from 2x to 1x mode and
# its ~1.9us ops head-of-line-block the PE queue)
GP_COLS = int(os.environ.get("K_GPCOLS", "0"))
# tiles contributing to the early (overlapped) AllGather
PHASE_A_TILES = int(os.environ.get("K_PHASEA", "40"))
NDIAG = 8


def _build(rows_per_core=ROWS_PER_CORE):
    nt_total = rows_per_core // 128
    pa = PHASE_A_TILES
    nc = bacc.Bacc("TRN2", target_bir_lowering=False, debug=False,
                   num_devices=N_CORES)
    X = nc.dram_tensor("X", [rows_per_core, P], BF16, kind="ExternalInput")
    MU = nc.dram_tensor("mu", [1, 5 * P], F32, kind="ExternalInput")
    W = nc.dram_tensor("W", [5 * P, K_OUT], F32, kind="ExternalInput")
    OUT = nc.dram_tensor("out", [1, K_OUT], F32, kind="ExternalOutput")

    # phase A payload: [S1|S2|S3|S5] raw sums over tiles [0, pa)
    ccA_in = nc.dram_tensor("ccA_in", [4 * P], F32)
    ccA_out = nc.dram_tensor("ccA_out", [N_CORES * 4 * P], F32,
                             addr_space="Shared")
    # phase B payload: [S1|S2|S3|S5|S4] (S4 over ALL tiles, rest over
    # tiles [pa, nt_total))
    ccB_in = nc.dram_tensor("ccB_in", [5 * P], F32)
    ccB_out = nc.dram_tensor("ccB_out", [N_CORES * 5 * P], F32,
                             addr_space="Shared")
    warm_in = nc.dram_tensor("warm_in", [5 * P], F32)
    warm_out = nc.dram_tensor("warm_out", [N_CORES * 5 * P], F32,
                              addr_space="Shared")

    # DMA blocks of 4 tiles; compute sub-blocks of 2 tiles
    dbs = [(i, min(4, nt_total - i)) for i in range(0, nt_total, 4)]

    with tile.TileContext(nc) as tc:
        with (
            tc.tile_pool(name="xin", bufs=6) as xin,
            tc.tile_pool(name="pows", bufs=4) as pows,
            tc.tile_pool(name="const", bufs=1) as const,
            tc.tile_pool(name="acc", bufs=1, space="PSUM") as accp,
            tc.tile_pool(name="tailps", bufs=1, space="PSUM") as tailps,
            tc.tile_pool(name="tail", bufs=1) as tailp,
        ):
            Xv = X.ap()

            # issue the first X block's DMA before any constant setup so
            # the HBM stream starts immediately
            t0_0, ndt_0 = dbs[0]
            x_first = xin.tile([128, ndt_0, P], BF16, tag="x")
            nh0 = max(1, ndt_0 // 2)
            for lo in range(0, ndt_0, nh0):
                w = min(nh0, ndt_0 - lo)
                nc.sync.dma_start(
                    out=x_first[:, lo:lo + w, :],
                    in_=Xv[(t0_0 + lo) * 128:(t0_0 + lo + w) * 128, :].rearrange(
                        "(p s) c -> p s c", s=w),
                )

            ones = const.tile([128, 1], BF16)
            nc.vector.memset(ones, 1.0)
            ident = const.tile([128, 128], F32)
            make_identity(nc, ident)
            # identity replicated along free axis for one-shot S4 extract
            ident_rep = const.tile([128, NDIAG, 128], F32)
            for c in range(NDIAG):
                nc.vector.tensor_copy(ident_rep[:, c, :], ident)

            # warm-up collective at the real payload size, launched
            # immediately: absorbs the ncfw/TOPSP cold-start barrier.
            wtile = const.tile([1, 8], F32)
            nc.vector.memset(wtile, 0.0)
            nc.scalar.dma_start(out=warm_in.ap()[0:8], in_=wtile[0:1, :])
            nc.gpsimd.collective_compute(
                "AllGather", Alu.bypass,
                replica_groups=[list(range(N_CORES))],
                ins=[warm_in.ap()],
                outs=[warm_out.ap()],
            )

            # weights / mu for the tail (contraction row j5 = 40*p + q)
            w_s = const.tile([128, 40, K_OUT], F32)
            nc.scalar.dma_start(out=w_s, in_=W.ap().rearrange(
                "(p q) k -> p q k", p=128))
            mu_s = const.tile([128, 40], F32)
            nc.scalar.dma_start(out=mu_s, in_=MU.ap()[0, :].rearrange(
                "(p q) -> p q", p=128))

            # PSUM accumulators, alive across the whole main loop.
            # ps_plain: S1@p0 S2@p32 S3@p64 S5@p96, reused by both phases
            # (phase A is DMA'd out mid-loop, then phase B restarts with
            # start=True). ps_diag accumulates S4 over ALL tiles.
            ps_plain = accp.tile([128, P], F32)
            ps_diag = accp.tile([128, NDIAG * 128], F32)

            for bi, (t0, ndt) in enumerate(dbs):
                if bi == 0:
                    x = x_first
                else:
                    x = xin.tile([128, ndt, P], BF16, tag="x")
                    # partition p holds CONSECUTIVE rows -> contiguous
                    # multi-KB DMA runs per partition (row->partition
                    # placement is free for column sums). Two DMAs per
                    # block for finer arrival.
                    nh = max(1, ndt // 2)
                    for lo in range(0, ndt, nh):
                        w = min(nh, ndt - lo)
                        nc.sync.dma_start(
                            out=x[:, lo:lo + w, :],
                            in_=Xv[(t0 + lo) * 128:(t0 + lo + w) * 128, :].rearrange(
                                "(p s) c -> p s c", s=w),
                        )
                for half in range(0, ndt, 2):
                    nt = min(2, ndt - half)
                    xh = x[:, half:half + nt, :]
                    sq = pows.tile([128, nt, P], BF16, tag="sq")
                    nc.scalar.activation(sq, xh, Act.Square)
                    cu = pows.tile([128, nt, P], BF16, tag="cu")
                    nc.vector.tensor_mul(cu, xh, sq)
                    x5 = pows.tile([128, nt, P], BF16, tag="x5")
                    if GP_COLS:
                        dv = P - GP_COLS
                        nc.vector.tensor_mul(x5[:, :, :dv], sq[:, :, :dv],
                                             cu[:, :, :dv])
                        nc.gpsimd.tensor_mul(x5[:, :, dv:], sq[:, :, dv:],
                                             cu[:, :, dv:])
                    else:
                        nc.vector.tensor_mul(x5, sq, cu)

                    for t in range(nt):
                        gt = t0 + half + t
                        st = gt == 0
                        sp = gt == nt_total - 1
                        # S4 diag blocks x2_chunk^T @ x2_chunk: depend
                        # only on sq; single phase across the whole loop.
                        # start/stop per PSUM bank (chunks 0-3 -> bank 0,
                        # 4-7 -> bank 1).
                        for c in range(NDIAG):
                            cs = slice(c * 128, (c + 1) * 128)
                            nc.tensor.matmul(
                                ps_diag[:, cs], sq[:, t, cs], sq[:, t, cs],
                                start=st and c % 4 == 0,
                                stop=sp and (c == 3 or c == NDIAG - 1),
                                tile_position=(0, 0),
                            )
                        # plain col-sums: 4 powers on 4 concurrent
                        # col-strips; two accumulation phases.
                        stp = gt == 0 or gt == pa
                        spp = gt == pa - 1 or gt == nt_total - 1
                        for h in range(2):
                            sl = slice(h * 512, (h + 1) * 512)
                            for j, pw in enumerate((xh, sq, cu, x5)):
                                bp = 32 * j
                                nc.tensor.matmul(
                                    ps_plain[bp:bp + 1, sl], ones[:, 0:1],
                                    pw[:, t, sl],
                                    start=stp, stop=spp,
                                    tile_position=(0, bp),
                                )

                    # phase A readout + early AllGather: one strided
                    # PSUM -> SBUF copy (rows 0/32/64/96 -> 4 partitions)
                    # + one DMA, then the collective overlaps the rest
                    # of the main loop.
                    if t0 + half + nt == pa:
                        rowsA = tailp.tile([128, P], F32, name="rowsA")
                        nc.scalar.activation(rowsA, ps_plain, Act.Copy)
                        for jj in range(4):
                            nc.scalar.dma_start(
                                out=ccA_in.ap()[jj * P:(jj + 1) * P],
                                in_=rowsA[32 * jj:32 * jj + 1, :],
                            )
                        nc.gpsimd.collective_compute(
                            "AllGather", Alu.bypass,
                            replica_groups=[list(range(N_CORES))],
                            ins=[ccA_in.ap()],
                            outs=[ccA_out.ap()],
                        )

            # ---- tail ----
            # S4 diagonal extract: (ps_diag * ident_rep) then reduce the
            # 128-wide X axis -> (128, NDIAG) raw sums.
            s4_s = tailp.tile([128, NDIAG], F32)
            dummy = tailp.tile([128, NDIAG, 128], F32)
            nc.vector.scalar_tensor_tensor(
                dummy, ps_diag[:].rearrange("p (c i) -> p c i", i=128),
                1.0, ident_rep, Alu.mult, Alu.mult)
            nc.vector.tensor_reduce(
                s4_s, dummy, axis=mybir.AxisListType.X, op=Alu.add)

            # phase B payload: plain rows via one full-tile copy, S4 from SBUF
            rowsB = tailp.tile([128, P], F32, name="rowsB")
            nc.scalar.activation(rowsB, ps_plain, Act.Copy)
            for jj in range(4):
                eng = nc.scalar if jj % 2 == 0 else nc.sync
                eng.dma_start(
                    out=ccB_in.ap()[jj * P:(jj + 1) * P],
                    in_=rowsB[32 * jj:32 * jj + 1, :],
                )
            nc.sync.dma_start(
                out=ccB_in.ap()[4 * P:5 * P].rearrange("(c i) -> i c", i=128),
                in_=s4_s,
            )
            nc.gpsimd.collective_compute(
                "AllGather", Alu.bypass,
                replica_groups=[list(range(N_CORES))],
                ins=[ccB_in.ap()],
                outs=[ccB_out.ap()],
            )

            # gathered per-core partials -> summed raw moments.
            # momg*[p, k, core, cc]; column j of moment k is (8p + cc).
            # A-side gather + tree-sum depends only on the (long done)
            # phase-A collective, so it overlaps collective B's flight.
            ga = tailp.tile([128, 4, N_CORES, K_OUT], F32)
            gb = tailp.tile([128, 5, N_CORES, K_OUT], F32)
            ccAv = ccA_out.ap().rearrange(
                "(r k p c) -> p k r c", r=N_CORES, k=4, p=128)
            ccBv = ccB_out.ap().rearrange(
                "(r k p c) -> p k r c", r=N_CORES, k=5, p=128)
            for k in range(4):
                eng = nc.sync if k % 2 == 0 else nc.scalar
                eng.dma_start(out=ga[:, k, :, :], in_=ccAv[:, k, :, :])
            nc.vector.tensor_add(ga[:, :, 0:4, :], ga[:, :, 0:4, :],
                                 ga[:, :, 4:8, :])
            nc.vector.tensor_add(ga[:, :, 0:2, :], ga[:, :, 0:2, :],
                                 ga[:, :, 2:4, :])
            nc.vector.tensor_add(ga[:, :, 0:1, :], ga[:, :, 0:1, :],
                                 ga[:, :, 1:2, :])
            for k in range(5):
                eng = nc.scalar if k % 2 == 0 else nc.sync
                eng.dma_start(out=gb[:, k, :, :], in_=ccBv[:, k, :, :])
            nc.vector.tensor_add(gb[:, :, 0:4, :], gb[:, :, 0:4, :],
                                 gb[:, :, 4:8, :])
            nc.vector.tensor_add(gb[:, :, 0:2, :], gb[:, :, 0:2, :],
                                 gb[:, :, 2:4, :])
            nc.vector.tensor_add(gb[:, :, 0:1, :], gb[:, :, 0:1, :],
                                 gb[:, :, 1:2, :])
            # smom[p, k, cc], k in [M1,M2,M3,M5,M4] order; scaled by 1/N
            smom = tailp.tile([128, 5, K_OUT], F32)
            nc.vector.tensor_add(smom[:, 0:4, :], ga[:, :, 0, :],
                                 gb[:, 0:4, 0, :])
            nc.vector.tensor_copy(smom[:, 4, :], gb[:, 4, 0, :])
            nc.vector.tensor_scalar_mul(smom, smom, SCALE)

            m = smom[:, 0, :]
            M2 = smom[:, 1, :]
            M3 = smom[:, 2, :]
            M5 = smom[:, 3, :]
            M4 = smom[:, 4, :]

            stt = nc.vector.scalar_tensor_tensor
            scr = tailp.tile([128, 12, 8], F32)  # scratch (128,8) slots
            m2, m3, m5, a2, a3, a4, mu2, mu3, b1, c3, c4, t1 = (
                scr[:, i, :] for i in range(12))

            # cumulants written straight into interleaved v slices:
            # v[p, 5*cc + k] = c_k(col 8p+cc)
            v = tailp.tile([128, 40], F32)
            vv = v[:].rearrange("p (c k) -> p c k", k=5)

            nc.vector.tensor_mul(m2, m, m)                   # m^2
            nc.vector.tensor_mul(m3, m2, m)                  # m^3
            nc.vector.tensor_mul(m5, m2, m3)                 # m^5
            nc.vector.tensor_sub(mu2, M2, m2)                # mu2 = M2-m^2
            nc.vector.tensor_copy(vv[:, :, 0], m)
            nc.vector.memset(vv[:, :, 1], 0.0)
            nc.vector.tensor_copy(vv[:, :, 2], mu2)
            # mu3 = M3 + (-3 M2)*m + 2 m^3
            stt(b1, M2, -3.0, m, Alu.mult, Alu.mult)         # -3 m M2
            nc.vector.tensor_add(b1, b1, M3)
            stt(mu3, m3, 2.0, b1, Alu.mult, Alu.add)         # +2m^3
            # c3 = mu3 - 3 mu2^2
            stt(c3, mu2, -3.0, mu2, Alu.mult, Alu.mult)
            nc.vector.tensor_add(vv[:, :, 3], c3, mu3)
            # mu5 = M5 - 5 m M4 + 10 m^2 M3 - 10 m^3 M2 + 4 m^5
            stt(a4, M4, -5.0, m, Alu.mult, Alu.mult)
            stt(a3, M3, 10.0, m2, Alu.mult, Alu.mult)
            stt(a2, M2, -10.0, m3, Alu.mult, Alu.mult)
            nc.vector.tensor_add(a4, a4, M5)
            stt(a3, m5, 4.0, a3, Alu.mult, Alu.add)
            nc.vector.tensor_add(a4, a4, a3)
            nc.vector.tensor_add(a4, a4, a2)                 # mu5
            # c4 = mu5 - 10 mu2 mu3
            stt(t1, mu2, -10.0, mu3, Alu.mult, Alu.mult)
            nc.vector.tensor_add(vv[:, :, 4], a4, t1)

            nc.vector.tensor_sub(v, v, mu_s)

            # projection: collapse q on DVE (k-major scratch so one
            # X-axis reduce yields (128, K)), then a single matmul
            # collapses the partition axis.
            wv = w_s[:].rearrange("p q k -> p k q")
            prod = tailp.tile([128, K_OUT, 40], F32)
            for k in range(K_OUT):
                nc.vector.tensor_mul(prod[:, k, :], v, wv[:, k, :])
            colk = tailp.tile([128, K_OUT], F32)
            nc.vector.tensor_reduce(colk, prod,
                                    axis=mybir.AxisListType.X, op=Alu.add)
            ps_out = tailps.tile([1, K_OUT], F32)
            ones_f = tailp.tile([128, 1], F32)
            nc.vector.memset(ones_f, 1.0)
            nc.tensor.matmul(ps_out[0:1, :], ones_f[:, 0:1], colk,
                             start=True, stop=True)
            o_s = tailp.tile([1, K_OUT], F32)
            nc.vector.tensor_copy(o_s, ps_out)
            nc.sync.dma_start(out=OUT.ap(), in_=o_s)

    nc.compile()
    return nc


_NC = None


def _get_nc():
    global _NC
    if _NC is None:
        _NC = _build()
    return _NC


def _shard(X, mu, W):
    Xb = np.asarray(X, dtype=np.float32).astype(ml_dtypes.bfloat16)
    Xp = np.zeros((N_CORES * ROWS_PER_CORE, P), dtype=ml_dtypes.bfloat16)
    Xp[:Xb.shape[0]] = Xb
    return [
        {
            "X": np.ascontiguousarray(Xp[i * ROWS_PER_CORE:(i + 1) * ROWS_PER_CORE]),
            "mu": np.ascontiguousarray(mu.astype(np.float32)),
            "W": np.ascontiguousarray(W.astype(np.float32)),
        }
        for i in range(N_CORES)
    ]


def run(X, mu, W, trace=False, **trace_kwargs):
    nc = _get_nc()
    in_maps = _shard(X, np.asarray(mu), np.asarray(W))
    res = run_bass_kernel_spmd(nc, in_maps, core_ids=list(range(N_CORES)),
                               trace=trace, **trace_kwargs)
    return res


def kernel(X, mu, W):
    res = run(X, mu, W, trace=False)
    return np.asarray(res.results[0]["out"], dtype=np.float32)
